# revision 25
# baseline (speedup 1.0000x reference)
"""Trainium2 Bass kernel for AlignShouldersToXAxis.

Math: the reference's Rodrigues construction for aligning the frame-0
shoulder vector to +X collapses to a 2D rotation in the XY plane:

    dx, dy = (p_right - p_left).xy   (frame 0, joints 6/5)
    n  = sqrt(dx^2 + dy^2);  m = max(n, 1e-12)
    cx = dx/m, cy = dy/m
    valid = (n >= 1e-6) & (|cy| >= 1e-6)
    if not valid: R = I
    out_x = cx*x + cy*y ; out_y = -cy*x + cx*y ; out_z = z

The rotation matrix's third row/col is exactly identity, so the z
channel is a bit-exact passthrough -- the host copies it directly and
the device never sees it.  The x/y channels are shipped as fp16 planes
(the tolerance is 2e-2; fp16 keeps the max-normalized error ~1e-3),
while the frame-0 shoulder coordinates travel separately in full fp32
so the rotation scalars are computed at reference precision.

Per-core HBM traffic: 2 planes x 16 batches x 102400 pts x 2B = 6.55 MB
in + 6.55 MB out (vs 39.3 MB for the all-fp32 full-tensor variant).

Sharding: pure data parallel, batch dim 128 -> 8 cores x 16 batches.
Per-core layout: fp16 planes [16, 8, 2, 12800]; partition p = b*8+k
holds a contiguous 12800-pt chunk of batch b for both planes, and the
per-batch rotation scalars are per-partition values.
"""

import contextlib
import time

import numpy as np

import concourse.bacc as bacc
import concourse.mybir as mybir
from concourse.tile import TileContext
from concourse.bass_utils import run_bass_kernel_spmd

N_CORES = 8
B, T, J, C = 128, 4096, 25, 3
B_LOC = B // N_CORES            # 16 batches per core
PTS = T * J                     # 102400 points per batch per plane
K = 8                           # chunks per batch -> 16*8 = 128 partitions
N_PART = PTS // K               # 12800 points per partition per plane
F = 1600                        # points per partition per tile

EPS = 1e-6
_f32 = mybir.dt.float32
_f16 = mybir.dt.float16
_u8 = mybir.dt.uint8
UINT8_BIAS = 128.0   # HW fp16->uint8 DMA cast rounds (measured);
                     # CoreSim truncates, so mini shows ~2x our HW err


def build(b_loc=B_LOC, npts=N_PART, k=K, f=F, io_bufs=None, scr_bufs=3,
          tile_sizes=None, loop_r=None):
    """Build the per-core Bass program. Parameterized so tests can build a
    small variant for CoreSim, or a For_i-looped variant (loop_r) for the
    wall-clock slope benchmark."""
    if tile_sizes is None:
        if npts == N_PART:
            # swept in TimelineSim: 12 tiles of ~1072 beat 8x1600 by ~1.3us
            tile_sizes = [1072] * 11 + [1008]
        else:
            assert npts % f == 0
            tile_sizes = [f] * (npts // f)
    assert sum(tile_sizes) == npts
    n_tiles = len(tile_sizes)
    if io_bufs is None:
        io_bufs = n_tiles
    P = b_loc * k               # partitions used (128 in prod)
    assert P <= 128

    nc = bacc.Bacc("TRN2", target_bir_lowering=False, debug=False,
                   num_devices=N_CORES)
    xy = nc.dram_tensor("xy", [b_loc, k, 2, npts], _f16, kind="ExternalInput")
    sh = nc.dram_tensor("sh", [b_loc, 8], _f32, kind="ExternalInput")
    o = nc.dram_tensor("o", [b_loc, k, 2, npts], _u8, kind="ExternalOutput")
    xv = xy.rearrange("b k two n -> (b k) two n")
    ov = o.rearrange("b k two n -> (b k) two n")

    mult = mybir.AluOpType.mult
    add = mybir.AluOpType.add
    is_ge = mybir.AluOpType.is_ge

    with TileContext(nc) as tc:
        with tc.tile_pool(name="scal", bufs=1) as scal, \
             tc.tile_pool(name="data", bufs=io_bufs) as data, \
             contextlib.ExitStack() as stack:
            if loop_r is not None:
                # slope-benchmark mode: run the whole body loop_r times
                stack.enter_context(tc.For_i(0, loop_r, 1))
            # --- per-batch rotation scalars, computed redundantly on all
            # partitions of each batch (DMA-broadcast of the fp32 shoulder
            # coords [x5, y5, x6, y6] plus the output quant scale 1/s_out).
            # This tiny DMA MUST be issued before the big tile loads: the
            # cost-model DMA resource drains in issue order, and the whole
            # compute pipeline waits on these scalars. ---
            s4 = scal.tile([P, 8], _f32)
            nc.sync.dma_start(
                out=s4[:],
                in_=sh[:, 0:8].unsqueeze(1).to_broadcast((b_loc, k, 8)))

            # Big tile loads right behind it so the DMA engines stream
            # continuously; the scalar prep below overlaps with them.
            tiles = []
            off = 0
            for fi in tile_sizes:
                tile_ = data.tile([P, 2, fi], _f16, tag="io")
                nc.sync.dma_start(out=tile_, in_=xv[:, :, off:off + fi])
                tiles.append((tile_, off, fi))
                off += fi

            # Rotation scalars with the output quant scale folded in:
            #   ccos = (valid ? cx : 1)/s_out ; csin = (valid ? cy : 0)/s_out
            # Critical chain (9 ops): sub -> sq -> nsq -> sqrt -> max ->
            # recip -> rs -> ccos/csin -> ncsin.  The validity branch runs
            # in parallel off the sqrt/recip path using squared thresholds:
            #   valid = (nsq >= EPS^2) & (dy^2 >= EPS^2 * nsq)
            invs = s4[:, 4:5]
            d2 = scal.tile([P, 2], _f32)      # (dx, dy)
            nc.vector.tensor_sub(d2, s4[:, 2:4], s4[:, 0:2])
            sq = scal.tile([P, 2], _f32)
            nc.vector.tensor_mul(sq, d2, d2)
            nsq = scal.tile([P, 1], _f32)
            nc.vector.tensor_add(nsq, sq[:, 0:1], sq[:, 1:2])
            n = scal.tile([P, 1], _f32)
            nc.scalar.sqrt(n, nsq)
            m = scal.tile([P, 1], _f32)
            nc.vector.tensor_scalar_max(m, n, 1e-12)
            r = scal.tile([P, 1], _f32)
            nc.vector.reciprocal(r, m)
            rs = scal.tile([P, 1], _f32)      # invs / m
            nc.vector.tensor_mul(rs, r, invs)
            # (csin, ncsin, ccos all hang off rs in parallel below)
            # validity branch (off the critical path)
            v1 = scal.tile([P, 1], _f32)
            nc.vector.tensor_scalar(v1, nsq, EPS * EPS, None, is_ge)
            e2 = scal.tile([P, 1], _f32)
            nc.vector.tensor_scalar(e2, nsq, EPS * EPS, None, mult)
            v2 = scal.tile([P, 1], _f32)
            nc.vector.tensor_tensor(v2, sq[:, 1:2], e2, is_ge)
            valid = scal.tile([P, 1], _f32)
            nc.vector.tensor_mul(valid, v1, v2)
            vd = scal.tile([P, 2], _f32)      # valid * (dx, dy)
            nc.vector.tensor_scalar(vd, d2, valid, None, mult)
            imn = scal.tile([P, 1], _f32)     # valid*invs - invs
            nc.vector.scalar_tensor_tensor(imn, valid, invs, invs,
                                           mult, mybir.AluOpType.subtract)
            # ccos = vd_x*rs - imn ; csin = vd_y*rs ; ncsin = -csin
            ccos = scal.tile([P, 1], _f32)
            nc.vector.scalar_tensor_tensor(ccos, vd[:, 0:1], rs, imn,
                                           mult, mybir.AluOpType.subtract)
            csin = scal.tile([P, 1], _f32)
            nc.vector.tensor_scalar(csin, vd[:, 1:2], rs, None, mult)
            ncsin = scal.tile([P, 1], _f32)
            nc.vector.tensor_scalar(ncsin, vd[:, 1:2], rs, -1.0, mult, mult)

            # --- streaming rotate: in-place on the IO tile, then an fp16 ->
            # int8 cast-store through SWDGE.  Work split so every engine fits
            # the per-tile DMA cadence: ACT does one mul (~1.5us); DVE does
            # two 4x-mode tensor_scalar muls (t_cy, whole-tile ccos scale)
            # plus two 2x-mode tensor_tensor adds. ---
            # The uint8 bias 128.5 rides on the cross terms: the SWDGE
            # fp16->uint8 cast truncates, and truncation of (q + 128.5)
            # is exactly round-half-up of q, offset by 128.
            #
            # Per tile: cross terms into one contiguous [P, 2, f] scratch
            # (t_c[0] = cy*y/s + b pairs with the x plane, t_c[1] =
            # -cy*x/s + b with the y plane), then ONE whole-tile 4x-mode
            # scale by cx/s and ONE whole-tile 2x-mode add.  t_cx always
            # runs on ACT; t_cy alternates DVE/ACT to balance the spans.
            copy_fn = mybir.ActivationFunctionType.Copy
            ubias = UINT8_BIAS
            for ti, (tile_, off, fi) in enumerate(tiles):
                xw = tile_[:, 0, :]
                yw = tile_[:, 1, :]
                t_c = data.tile([P, 2, fi], _f16, tag="t_c", bufs=scr_bufs)
                nc.scalar.activation(t_c[:, 1, :], xw, copy_fn, bias=ubias,
                                     scale=ncsin)
                if ti % 2 == 1:
                    nc.scalar.activation(t_c[:, 0, :], yw, copy_fn,
                                         bias=ubias, scale=csin)
                else:
                    nc.vector.tensor_scalar(t_c[:, 0, :], yw, csin, ubias,
                                            mult, add)
                flat2 = tile_.rearrange("p two n -> p (two n)")
                tcf = t_c.rearrange("p two n -> p (two n)")
                nc.vector.tensor_scalar(flat2, flat2, ccos, None, mult)
                nc.vector.tensor_add(flat2, flat2, tcf)
                nc.gpsimd.dma_start(out=ov[:, :, off:off + fi], in_=tile_)
    nc.compile()
    return nc


_nc_cache = None


def kernel(skeleton_seq: np.ndarray) -> np.ndarray:
    global _nc_cache
    skeleton_seq = np.asarray(skeleton_seq)
    assert skeleton_seq.shape == (B, T, J, C), skeleton_seq.shape
    if _nc_cache is None:
        _nc_cache = build()
    nc = _nc_cache

    v = np.ascontiguousarray(skeleton_seq, dtype=np.float32).reshape(B, PTS, C)
    # fp16 x/y planes, chunk-major: [B, K, 2, N_PART]
    xy16 = np.empty((B, K, 2, N_PART), dtype=np.float16)
    xy16[:, :, 0, :] = v[:, :, 0].reshape(B, K, N_PART)
    xy16[:, :, 1, :] = v[:, :, 1].reshape(B, K, N_PART)
    # Output int8 quantization scale.  The rotation preserves the xy pair
    # norm, so |x'|,|y'| <= max_b,t,j ||(x,y)||_2 =: p_max exactly, and a
    # grid of p_max/127 can never saturate.
    p2 = 0.0
    for b in range(B):
        vb = v[b]
        p2 = max(p2, float((vb[:, 0] ** 2 + vb[:, 1] ** 2).max()))
    s_out = np.sqrt(p2) / 127.0 if p2 > 0.0 else 1.0
    # frame-0 shoulder coords in full fp32 + folded quant scale
    shf = np.zeros((B, 8), dtype=np.float32)
    shf[:, 0:2] = v[:, 5, 0:2]
    shf[:, 2:4] = v[:, 6, 0:2]
    shf[:, 4] = 1.0 / s_out

    in_maps = [
        {"xy": xy16[i * B_LOC:(i + 1) * B_LOC],
         "sh": shf[i * B_LOC:(i + 1) * B_LOC]}
        for i in range(N_CORES)
    ]
    # The axon-tunneled devices occasionally throw a transient
    # NRT_EXEC_UNIT_UNRECOVERABLE on the first execution after another
    # process released them; retry before giving up.
    last_err = None
    for attempt in range(3):
        try:
            res = run_bass_kernel_spmd(nc, in_maps,
                                       core_ids=list(range(N_CORES)))
            break
        except Exception as e:  # noqa: BLE001
            last_err = e
            time.sleep(5.0 * (attempt + 1))
    else:
        raise last_err

    out = np.empty((B, PTS, C), dtype=np.float32)
    sf = np.float32(s_out)
    for i in range(N_CORES):
        oi = res.results[i]["o"]            # [B_LOC, K, 2, N_PART] int8
        out[i * B_LOC:(i + 1) * B_LOC, :, 0] = \
            (oi[:, :, 0, :].reshape(B_LOC, PTS).astype(np.float32) - 128.0) * sf
        out[i * B_LOC:(i + 1) * B_LOC, :, 1] = \
            (oi[:, :, 1, :].reshape(B_LOC, PTS).astype(np.float32) - 128.0) * sf
    out[:, :, 2] = v[:, :, 2]
    return out.reshape(B, T, J, C)


# revision 27
# speedup vs baseline: 1.0119x; 1.0119x over previous
"""Trainium2 Bass kernel for AlignShouldersToXAxis.

Math: the reference's Rodrigues construction for aligning the frame-0
shoulder vector to +X collapses to a 2D rotation in the XY plane:

    dx, dy = (p_right - p_left).xy   (frame 0, joints 6/5)
    n  = sqrt(dx^2 + dy^2);  m = max(n, 1e-12)
    cx = dx/m, cy = dy/m
    valid = (n >= 1e-6) & (|cy| >= 1e-6)
    if not valid: R = I
    out_x = cx*x + cy*y ; out_y = -cy*x + cx*y ; out_z = z

The rotation matrix's third row/col is exactly identity, so the z
channel is a bit-exact passthrough -- the host copies it directly and
the device never sees it.  The x/y channels are shipped as fp16 planes
(the tolerance is 2e-2; fp16 keeps the max-normalized error ~1e-3),
while the frame-0 shoulder coordinates travel separately in full fp32
so the rotation scalars are computed at reference precision.

Per-core HBM traffic: 6.55 MB fp16 in + 3.28 MB uint8 out (the output
rides an int8 grid of p_max/127 where p_max is the max xy pair norm --
the rotation preserves pair norms, so the grid can never saturate; the
host de-quantizes).  vs 39.3 MB/core for the all-fp32 full-tensor
variant.

Sharding: pure data parallel, batch dim 128 -> 8 cores x 16 batches.
Per-core layout: fp16 planes [16, 8, 2, 12800]; partition p = b*8+k
holds a contiguous 12800-pt chunk of batch b for both planes, and the
per-batch rotation scalars are per-partition values.
"""

import contextlib
import time

import numpy as np

import concourse.bacc as bacc
import concourse.mybir as mybir
from concourse.tile import TileContext
from concourse.bass_utils import run_bass_kernel_spmd

N_CORES = 8
B, T, J, C = 128, 4096, 25, 3
B_LOC = B // N_CORES            # 16 batches per core
PTS = T * J                     # 102400 points per batch per plane
K = 8                           # chunks per batch -> 16*8 = 128 partitions
N_PART = PTS // K               # 12800 points per partition per plane
F = 1600                        # points per partition per tile

EPS = 1e-6
_f32 = mybir.dt.float32
_f16 = mybir.dt.float16
_u8 = mybir.dt.uint8
UINT8_BIAS = 128.0   # HW fp16->uint8 DMA cast rounds (measured);
                     # CoreSim truncates, so mini shows ~2x our HW err


def build(b_loc=B_LOC, npts=N_PART, k=K, f=F, io_bufs=None, scr_bufs=3,
          tile_sizes=None, loop_r=None):
    """Build the per-core Bass program. Parameterized so tests can build a
    small variant for CoreSim, or a For_i-looped variant (loop_r) for the
    wall-clock slope benchmark."""
    if tile_sizes is None:
        if npts == N_PART:
            # swept in TimelineSim: 12 tiles, slight taper at the end
            tile_sizes = [1120] * 10 + [832, 768]
        else:
            assert npts % f == 0
            tile_sizes = [f] * (npts // f)
    assert sum(tile_sizes) == npts
    n_tiles = len(tile_sizes)
    if io_bufs is None:
        io_bufs = n_tiles
    P = b_loc * k               # partitions used (128 in prod)
    assert P <= 128

    nc = bacc.Bacc("TRN2", target_bir_lowering=False, debug=False,
                   num_devices=N_CORES)
    xy = nc.dram_tensor("xy", [b_loc, k, 2, npts], _f16, kind="ExternalInput")
    sh = nc.dram_tensor("sh", [b_loc, 8], _f32, kind="ExternalInput")
    o = nc.dram_tensor("o", [b_loc, k, 2, npts], _u8, kind="ExternalOutput")
    xv = xy.rearrange("b k two n -> (b k) two n")
    ov = o.rearrange("b k two n -> (b k) two n")

    mult = mybir.AluOpType.mult
    add = mybir.AluOpType.add
    is_ge = mybir.AluOpType.is_ge

    with TileContext(nc) as tc:
        with tc.tile_pool(name="scal", bufs=1) as scal, \
             tc.tile_pool(name="data", bufs=io_bufs) as data, \
             contextlib.ExitStack() as stack:
            if loop_r is not None:
                # slope-benchmark mode: run the whole body loop_r times
                stack.enter_context(tc.For_i(0, loop_r, 1))
            # --- per-batch rotation scalars, computed redundantly on all
            # partitions of each batch (DMA-broadcast of the fp32 shoulder
            # coords [x5, y5, x6, y6] plus the output quant scale 1/s_out).
            # This tiny DMA MUST be issued before the big tile loads: the
            # cost-model DMA resource drains in issue order, and the whole
            # compute pipeline waits on these scalars. ---
            s4 = scal.tile([P, 8], _f32)
            nc.gpsimd.dma_start(
                out=s4[:],
                in_=sh[:, 0:8].unsqueeze(1).to_broadcast((b_loc, k, 8)))

            # Big tile loads right behind it so the DMA engines stream
            # continuously; the scalar prep below overlaps with them.
            tiles = []
            off = 0
            for fi in tile_sizes:
                tile_ = data.tile([P, 2, fi], _f16, tag="io")
                nc.sync.dma_start(out=tile_, in_=xv[:, :, off:off + fi])
                tiles.append((tile_, off, fi))
                off += fi

            # Rotation scalars with the output quant scale folded in:
            #   ccos = (valid ? cx : 1)/s_out ; csin = (valid ? cy : 0)/s_out
            # Critical chain (9 ops): sub -> sq -> nsq -> sqrt -> max ->
            # recip -> rs -> ccos/csin -> ncsin.  The validity branch runs
            # in parallel off the sqrt/recip path using squared thresholds:
            #   valid = (nsq >= EPS^2) & (dy^2 >= EPS^2 * nsq)
            invs = s4[:, 4:5]
            d2 = scal.tile([P, 2], _f32)      # (dx, dy)
            nc.vector.tensor_sub(d2, s4[:, 2:4], s4[:, 0:2])
            sq = scal.tile([P, 2], _f32)
            nc.vector.tensor_mul(sq, d2, d2)
            nsq = scal.tile([P, 1], _f32)
            nc.vector.tensor_add(nsq, sq[:, 0:1], sq[:, 1:2])
            n = scal.tile([P, 1], _f32)
            nc.scalar.sqrt(n, nsq)
            m = scal.tile([P, 1], _f32)
            nc.vector.tensor_scalar_max(m, n, 1e-12)
            r = scal.tile([P, 1], _f32)
            nc.vector.reciprocal(r, m)
            rs = scal.tile([P, 1], _f32)      # invs / m
            nc.vector.tensor_mul(rs, r, invs)
            # (csin, ncsin, ccos all hang off rs in parallel below)
            # validity branch (off the critical path)
            v1 = scal.tile([P, 1], _f32)
            nc.vector.tensor_scalar(v1, nsq, EPS * EPS, None, is_ge)
            e2 = scal.tile([P, 1], _f32)
            nc.vector.tensor_scalar(e2, nsq, EPS * EPS, None, mult)
            v2 = scal.tile([P, 1], _f32)
            nc.vector.tensor_tensor(v2, sq[:, 1:2], e2, is_ge)
            valid = scal.tile([P, 1], _f32)
            nc.vector.tensor_mul(valid, v1, v2)
            vd = scal.tile([P, 2], _f32)      # valid * (dx, dy)
            nc.vector.tensor_scalar(vd, d2, valid, None, mult)
            imn = scal.tile([P, 1], _f32)     # valid*invs - invs
            nc.vector.scalar_tensor_tensor(imn, valid, invs, invs,
                                           mult, mybir.AluOpType.subtract)
            # ccos = vd_x*rs - imn ; csin = vd_y*rs ; ncsin = -csin
            ccos = scal.tile([P, 1], _f32)
            nc.vector.scalar_tensor_tensor(ccos, vd[:, 0:1], rs, imn,
                                           mult, mybir.AluOpType.subtract)
            csin = scal.tile([P, 1], _f32)
            nc.vector.tensor_scalar(csin, vd[:, 1:2], rs, None, mult)
            ncsin = scal.tile([P, 1], _f32)
            nc.vector.tensor_scalar(ncsin, vd[:, 1:2], rs, -1.0, mult, mult)

            # --- streaming rotate: in-place on the IO tile, then an fp16 ->
            # int8 cast-store through SWDGE.  Work split so every engine fits
            # the per-tile DMA cadence: ACT does one mul (~1.5us); DVE does
            # two 4x-mode tensor_scalar muls (t_cy, whole-tile ccos scale)
            # plus two 2x-mode tensor_tensor adds. ---
            # The uint8 bias 128.5 rides on the cross terms: the SWDGE
            # fp16->uint8 cast truncates, and truncation of (q + 128.5)
            # is exactly round-half-up of q, offset by 128.
            #
            # Per tile: cross terms into one contiguous [P, 2, f] scratch
            # (t_c[0] = cy*y/s + b pairs with the x plane, t_c[1] =
            # -cy*x/s + b with the y plane), then ONE whole-tile 4x-mode
            # scale by cx/s and ONE whole-tile 2x-mode add.  t_cx always
            # runs on ACT; t_cy runs on ACT for 2 of every 3 tiles (swept).
            copy_fn = mybir.ActivationFunctionType.Copy
            ubias = UINT8_BIAS
            for ti, (tile_, off, fi) in enumerate(tiles):
                xw = tile_[:, 0, :]
                yw = tile_[:, 1, :]
                t_c = data.tile([P, 2, fi], _f16, tag="t_c", bufs=scr_bufs)
                nc.scalar.activation(t_c[:, 1, :], xw, copy_fn, bias=ubias,
                                     scale=ncsin)
                if ti % 3 != 0:
                    nc.scalar.activation(t_c[:, 0, :], yw, copy_fn,
                                         bias=ubias, scale=csin)
                else:
                    nc.vector.tensor_scalar(t_c[:, 0, :], yw, csin, ubias,
                                            mult, add)
                flat2 = tile_.rearrange("p two n -> p (two n)")
                tcf = t_c.rearrange("p two n -> p (two n)")
                nc.vector.tensor_scalar(flat2, flat2, ccos, None, mult)
                nc.vector.tensor_add(flat2, flat2, tcf)
                nc.gpsimd.dma_start(out=ov[:, :, off:off + fi], in_=tile_)
    nc.compile()
    return nc


_nc_cache = None


def kernel(skeleton_seq: np.ndarray) -> np.ndarray:
    global _nc_cache
    skeleton_seq = np.asarray(skeleton_seq)
    assert skeleton_seq.shape == (B, T, J, C), skeleton_seq.shape
    if _nc_cache is None:
        _nc_cache = build()
    nc = _nc_cache

    v = np.ascontiguousarray(skeleton_seq, dtype=np.float32).reshape(B, PTS, C)
    # fp16 x/y planes, chunk-major: [B, K, 2, N_PART]
    xy16 = np.empty((B, K, 2, N_PART), dtype=np.float16)
    xy16[:, :, 0, :] = v[:, :, 0].reshape(B, K, N_PART)
    xy16[:, :, 1, :] = v[:, :, 1].reshape(B, K, N_PART)
    # Output int8 quantization scale.  The rotation preserves the xy pair
    # norm, so |x'|,|y'| <= max_b,t,j ||(x,y)||_2 =: p_max exactly, and a
    # grid of p_max/127 can never saturate.
    p2 = 0.0
    for b in range(B):
        vb = v[b]
        p2 = max(p2, float((vb[:, 0] ** 2 + vb[:, 1] ** 2).max()))
    s_out = np.sqrt(p2) / 127.0 if p2 > 0.0 else 1.0
    # frame-0 shoulder coords in full fp32 + folded quant scale
    shf = np.zeros((B, 8), dtype=np.float32)
    shf[:, 0:2] = v[:, 5, 0:2]
    shf[:, 2:4] = v[:, 6, 0:2]
    shf[:, 4] = 1.0 / s_out

    in_maps = [
        {"xy": xy16[i * B_LOC:(i + 1) * B_LOC],
         "sh": shf[i * B_LOC:(i + 1) * B_LOC]}
        for i in range(N_CORES)
    ]
    # The axon-tunneled devices occasionally throw a transient
    # NRT_EXEC_UNIT_UNRECOVERABLE on the first execution after another
    # process released them; retry before giving up.
    last_err = None
    for attempt in range(3):
        try:
            res = run_bass_kernel_spmd(nc, in_maps,
                                       core_ids=list(range(N_CORES)))
            break
        except Exception as e:  # noqa: BLE001
            last_err = e
            time.sleep(5.0 * (attempt + 1))
    else:
        raise last_err

    out = np.empty((B, PTS, C), dtype=np.float32)
    sf = np.float32(s_out)
    for i in range(N_CORES):
        oi = res.results[i]["o"]            # [B_LOC, K, 2, N_PART] int8
        out[i * B_LOC:(i + 1) * B_LOC, :, 0] = \
            (oi[:, :, 0, :].reshape(B_LOC, PTS).astype(np.float32) - 128.0) * sf
        out[i * B_LOC:(i + 1) * B_LOC, :, 1] = \
            (oi[:, :, 1, :].reshape(B_LOC, PTS).astype(np.float32) - 128.0) * sf
    out[:, :, 2] = v[:, :, 2]
    return out.reshape(B, T, J, C)
